# revision 96
# baseline (speedup 1.0000x reference)
"""AttentionBlock Trainium2 Bass kernel (fp8 DoubleRow redesign).

Data-parallel over batch: 16 batches / 8 cores = 2 per core.

Key design points vs the fp32r baseline:
- fp8e4m3 DoubleRow matmuls (contraction 256/instr, 0.5 cyc/row) for the
  q/k/v projections, QK scores, and AV; fp32r only for the output
  projection (accuracy).
- Weight pre-scaling: Wq,Wk x16 (compensated by exp scale 1/256), Wv x16
  (compensated by Wp/16) keeps fp8 weight entries out of the subnormal
  range.
- QK packs 4 heads' k-channels (256) into one DoubleRow contraction with
  per-head zero-padded q operands (zeros memset once, rows rewritten per
  batch).
- exp split across engines: Act does native exp -> fp8 wts; DVE/Pool do
  Schraudolph bf16 exp (int16 bit trick) for a subset of seq-chunks whose
  AV runs in bf16.
- softmax denominator from a ones-column appended to vT (row 64 of the AV
  psum); reciprocal on DVE, broadcast via a tiny PE matmul (ones x recip)
  into PSUM, normalize fused with the PSUM->SBUF move.
- groupnorm stats via bn_stats on bf16 inputs + block-diag matmul
  reduction; rsqrt via quake seed + 2 Newton steps (keeps Act on exp).
- x/y staged in bf16 (halves DMA); residual add reuses the staged x.
"""
import os
import sys

sys.path.insert(0, "/opt/trn_rl_repo")

import numpy as np

import concourse.bacc as bacc
import concourse.bass as bass
import concourse.tile as tile
from concourse import mybir
from concourse.bass_utils import run_bass_kernel_spmd

F32 = mybir.dt.float32
F32R = mybir.dt.float32r
BF16 = mybir.dt.bfloat16
FP8 = mybir.dt.float8e4
I16 = mybir.dt.int16
I32 = mybir.dt.int32
AF = mybir.ActivationFunctionType
OP = mybir.AluOpType
PM = mybir.MatmulPerfMode

B, C, H, W = 16, 512, 32, 32
T = H * W              # 1024
NH = 8                 # heads
CH = C // NH           # 64
GROUPS = 32
GSIZE = C // GROUPS    # 16
EPS = 1e-5
N_CORES = 8
BPC = B // N_CORES     # batches per core
CB = C // 128          # 4 channel blocks
NT = T // 512          # 2 column halves
ST = T // 128          # 8 seq tiles of 128
NK = ST // 2           # 4 DoubleRow seq chunks of 256

WSCALE = 16.0          # fp8 weight pre-scale for Wq/Wk/Wv
QK_SCALE = 1.0 / (WSCALE * WSCALE)   # folded into exp
# softmax shift: wts = exp(logit - EXP_SHIFT); cancels in normalization and
# keeps the max weight well under the fp8e4m3 max (448)
EXP_SHIFT = 2.0
# Schraudolph bf16 exp: bits16 = x*(2^7/ln2)*QK_SCALE + (127*2^7 - c)
SCH_A = 184.66496580927726 * QK_SCALE
SCH_B = 16250.4 - EXP_SHIFT * 184.66496580927726

# exp engine per k-chunk (0..3), by (batch, head parity): 'A' = Act native
# exp (fp8 wts, DR AV), 'D' = DVE Schraudolph (bf16 wts, bf16 AV). GPSIMD
# can't touch PSUM, so Pool gets only SBUF work. Batch 1 has no background
# work to hide DVE latency behind, so it runs all-Act.
EXP_ENG_B = {
    0: {0: ['A', 'D', 'A', 'A'], 1: ['A', 'D', 'A', 'A']},
    1: {0: ['A', 'A', 'A', 'A'], 1: ['A', 'A', 'A', 'A']},
}


def _build():
    nc = bacc.Bacc(None, target_bir_lowering=False)

    x2 = nc.dram_tensor("x2", (BPC, C, T), BF16, kind="ExternalInput")
    y2 = nc.dram_tensor("y2", (BPC, C, T), BF16, kind="ExternalInput")
    wq8_d = nc.dram_tensor("wq8", (128, 2, 2, C), FP8, kind="ExternalInput")
    wk8_d = nc.dram_tensor("wk8", (128, 2, 2, C), FP8, kind="ExternalInput")
    wv8_d = nc.dram_tensor("wv8", (128, 2, 2, C), FP8, kind="ExternalInput")
    wpt = nc.dram_tensor("wpt", (C, C), F32, kind="ExternalInput")
    bq_l = nc.dram_tensor("bq_l", (128, CB), F32, kind="ExternalInput")
    bk_l = nc.dram_tensor("bk_l", (128, CB), F32, kind="ExternalInput")
    bp_l = nc.dram_tensor("bp_l", (128, CB), F32, kind="ExternalInput")
    bv_bc = nc.dram_tensor("bv_bc", (128, NH, CH), F32, kind="ExternalInput")
    gnw_l = nc.dram_tensor("gnw_l", (128, CB), F32, kind="ExternalInput")
    gnb_l = nc.dram_tensor("gnb_l", (128, CB), F32, kind="ExternalInput")
    m1 = nc.dram_tensor("m1", (128, 128), F32, kind="ExternalInput")
    vcap8_d = nc.dram_tensor("vcap8", (128, NH, 2), FP8, kind="ExternalInput")
    vcap16_d = nc.dram_tensor("vcap16", (128, NH, 2), BF16, kind="ExternalInput")
    ident16_d = nc.dram_tensor("ident16", (128, 128), BF16, kind="ExternalInput")
    out_d = nc.dram_tensor("out", (BPC, C, T), F32, kind="ExternalOutput")
    DEBUG = bool(int(os.environ.get("KERNEL_DEBUG", "0")))
    if DEBUG:
        dbg_gnx = nc.dram_tensor("dbg_gnx", (2, 128, 2, T), F32, kind="ExternalOutput")
        dbg_kq = nc.dram_tensor("dbg_kq", (2, 128, 2, T), F32, kind="ExternalOutput")
        dbg_qp = nc.dram_tensor("dbg_qp", (2, 128, 2, T), F32, kind="ExternalOutput")
        dbg_a = nc.dram_tensor("dbg_a", (128, CB, T), F32, kind="ExternalOutput")
        dbg_r = nc.dram_tensor("dbg_r", (1, T), F32, kind="ExternalOutput")
        dbg_w = nc.dram_tensor("dbg_w", (NK, 128, 2, T), F32, kind="ExternalOutput")

    with tile.TileContext(nc) as tc:
        from contextlib import ExitStack
        with ExitStack() as ctx:
            consts = ctx.enter_context(tc.tile_pool(name="consts", bufs=1))
            px = ctx.enter_context(tc.tile_pool(name="px", bufs=2))
            py = ctx.enter_context(tc.tile_pool(name="py", bufs=1))
            pgn = ctx.enter_context(tc.tile_pool(name="pgn", bufs=3))
            pkq = ctx.enter_context(tc.tile_pool(name="pkq", bufs=4))
            pvt8 = ctx.enter_context(tc.tile_pool(name="pvt8", bufs=7))
            pvt16 = ctx.enter_context(tc.tile_pool(name="pvt16", bufs=2))
            pw8 = ctx.enter_context(tc.tile_pool(name="pw8", bufs=6))
            pw16 = ctx.enter_context(tc.tile_pool(name="pw16", bufs=3))
            pa = ctx.enter_context(tc.tile_pool(name="pa", bufs=2))
            pr0 = ctx.enter_context(tc.tile_pool(name="pr0", bufs=1))
            pst = ctx.enter_context(tc.tile_pool(name="pst", bufs=4))
            post = ctx.enter_context(tc.tile_pool(name="post", bufs=3))
            prbc = ctx.enter_context(tc.tile_pool(name="prbc", bufs=2))
            pdbg = ctx.enter_context(tc.tile_pool(name="pdbg", bufs=1)) if bool(int(os.environ.get("KERNEL_DEBUG", "0"))) else None
            ps_proj = ctx.enter_context(tc.tile_pool(name="ps_proj", bufs=2, space="PSUM"))
            ps_sc = ctx.enter_context(tc.tile_pool(name="ps_sc", bufs=2, space="PSUM"))
            ps_a = ctx.enter_context(tc.tile_pool(name="ps_a", bufs=1, space="PSUM"))

            # ---------------- constants ----------------
            wq_sb = consts.tile([128, 2, 2, C], FP8, tag="wq")
            wk_sb = consts.tile([128, 2, 2, C], FP8, tag="wk")
            wv_sb = consts.tile([128, 2, 2, C], FP8, tag="wv")
            wp_sb = consts.tile([128, CB, C], F32R, tag="wp")

            def emit_weight_loads():
                nc.sync.dma_start(out=wk_sb, in_=wk8_d[:, :, :, :])
                nc.sync.dma_start(out=wv_sb, in_=wv8_d[:, :, :, :])
                nc.sync.dma_start(out=wq_sb, in_=wq8_d[:, :, :, :])
                nc.sync.dma_start(out=wp_sb, in_=wpt.rearrange("(kb p) o -> p kb o", p=128).bitcast(F32R))

            m1_sb = consts.tile([128, 128], F32, tag="m1")
            bq_sb = consts.tile([128, CB], F32, tag="bq")
            bk_sb = consts.tile([128, CB], F32, tag="bk")
            bp_sb = consts.tile([128, CB], F32, tag="bp")
            bv_sb = consts.tile([128, NH, CH], F32, tag="bv")
            gnw_sb = consts.tile([128, CB], F32, tag="gnw")
            gnb_sb = consts.tile([128, CB], F32, tag="gnb")
            vcap8_sb = consts.tile([128, NH, 2], FP8, tag="vcap8")
            vcap16_sb = consts.tile([128, NH, 2], BF16, tag="vcap16")
            ident16_sb = consts.tile([128, 128], BF16, tag="ident16")
            magic_sb = consts.tile([128, CB], I32, tag="magic")
            nc.vector.memset(magic_sb, 0x5f3759df)
            shift_sb = consts.tile([128, 1], F32, tag="shift")
            nc.vector.memset(shift_sb, -EXP_SHIFT)
            warm = consts.tile([1, 1], F32, tag="warm")
            nc.vector.memset(warm, 0.0)
            nc.scalar.activation(out=warm, in_=warm, func=AF.Exp)

            def emit_small_consts():
                nc.sync.dma_start(out=m1_sb, in_=m1[:, :])
                nc.sync.dma_start(out=gnw_sb, in_=gnw_l[:, :])
                nc.sync.dma_start(out=gnb_sb, in_=gnb_l[:, :])
                nc.sync.dma_start(out=bk_sb, in_=bk_l[:, :])
                nc.sync.dma_start(out=bq_sb, in_=bq_l[:, :])
                nc.sync.dma_start(out=bv_sb, in_=bv_bc[:, :, :])
                nc.sync.dma_start(out=bp_sb, in_=bp_l[:, :])
                nc.sync.dma_start(out=vcap8_sb, in_=vcap8_d[:, :, :])
                nc.sync.dma_start(out=vcap16_sb, in_=vcap16_d[:, :, :])
                nc.sync.dma_start(out=ident16_sb, in_=ident16_d[:, :])

            # persistent zero-padded q operands: [128, 2, T] fp8 per head,
            # double-buffered by batch parity (avoids WAR with the next
            # batch's q_proj); head h occupies rows (h%2)*64.. at sub
            # (h//2)%2; zeros persist.
            q_pad_sets = [
                [consts.tile([128, 2, T], FP8, tag=f"qpad{g}_{h}", name=f"qpad{g}_{h}")
                 for h in range(NH)]
                for g in range(2)]

            def emit_qpad_memsets(g):
                # Pool only (idle engine), chunked so ready work can slip in
                for qp in q_pad_sets[g]:
                    qpf = qp.rearrange("p a b -> p (a b)")
                    for c in range(4):
                        nc.gpsimd.memset(qpf[:, c * 512:(c + 1) * 512], 0.0)

            # ---------------- groupnorm ----------------
            def groupnorm(src_sb, gn_tiles, spread=False):
                """src_sb: [128, CB, T] bf16. gn_tiles: 2 x [128, 2, T] fp8."""
                mv = pst.tile([128, CB, 2], F32, tag="mv")
                stats6 = pst.tile([128, 2, 6], F32, tag="stats6")
                for cb in range(CB):
                    for c2 in range(2):
                        nc.vector.bn_stats(
                            out=stats6[:, c2, :],
                            in_=src_sb[:, cb, c2 * 512:(c2 + 1) * 512])
                    nc.vector.bn_aggr(out=mv[:, cb, :], in_=stats6)
                musq = pst.tile([128, 4], F32, tag="musq")
                nc.vector.tensor_tensor(out=musq, in0=mv[:, :, 0], in1=mv[:, :, 0], op=OP.mult)
                nc.vector.tensor_tensor(out=mv[:, :, 1], in0=musq, in1=mv[:, :, 1], op=OP.add)
                psgt = ps_proj.tile([128, 512], F32, tag="mm", name="psgt")
                psg = psgt[:, 0:8]
                nc.tensor.matmul(psg, m1_sb, mv.rearrange("p a b -> p (a b)"), start=True, stop=True)
                gsb = pst.tile([128, 8], F32, tag="gsb")
                nc.vector.tensor_copy(gsb, psg)
                tmp4 = pst.tile([128, 4], F32, tag="tmp4")
                nc.vector.tensor_tensor(out=tmp4, in0=gsb[:, 0::2], in1=gsb[:, 0::2], op=OP.mult)
                vv = pst.tile([128, 4], F32, tag="vv")
                nc.vector.scalar_tensor_tensor(
                    out=vv, in0=gsb[:, 1::2], scalar=EPS, in1=tmp4,
                    op0=OP.add, op1=OP.subtract)
                bsh = pst.tile([128, 4], I32, tag="bsh")
                nc.vector.tensor_scalar(
                    out=bsh, in0=vv.bitcast(I32), scalar1=1, scalar2=None,
                    op0=OP.logical_shift_right)
                nc.vector.tensor_tensor(out=tmp4.bitcast(I32), in0=magic_sb, in1=bsh, op=OP.subtract)
                nrt = pst.tile([128, 4], F32, tag="nrt")
                for _ in range(2):
                    nc.vector.tensor_tensor(out=nrt, in0=tmp4, in1=tmp4, op=OP.mult)
                    nc.vector.scalar_tensor_tensor(
                        out=nrt, in0=nrt, scalar=-0.5, in1=vv, op0=OP.mult, op1=OP.mult)
                    nc.vector.scalar_tensor_tensor(
                        out=tmp4, in0=nrt, scalar=1.5, in1=tmp4, op0=OP.add, op1=OP.mult)
                ab = pst.tile([128, 8], F32, tag="ab")
                nc.vector.tensor_tensor(out=ab[:, 0:4], in0=tmp4, in1=gnw_sb, op=OP.mult)
                tmp4b = pst.tile([128, 4], F32, tag="tmp4b")
                nc.vector.tensor_tensor(out=tmp4b, in0=gsb[:, 0::2], in1=ab[:, 0:4], op=OP.mult)
                nc.vector.tensor_tensor(out=ab[:, 4:8], in0=gnb_sb, in1=tmp4b, op=OP.subtract)
                for cb in range(CB):
                    kb2, i = cb // 2, cb % 2
                    for th in range(NT):
                        eng = nc.gpsimd if not spread else \
                            (nc.vector, nc.gpsimd)[(cb * NT + th) % 2]
                        eng.tensor_scalar(
                            out=gn_tiles[kb2][:, i, th * 512:(th + 1) * 512],
                            in0=src_sb[:, cb, th * 512:(th + 1) * 512],
                            scalar1=ab[:, cb:cb + 1], scalar2=ab[:, 4 + cb:5 + cb],
                            op0=OP.mult, op1=OP.add)

            def emit_load(b, dram, pool, tag):
                sb = pool.tile([128, CB, T], BF16, tag=tag, name=tag)
                for cb in range(CB):
                    deng = nc.sync if cb % 2 == 0 else nc.scalar
                    deng.dma_start(
                        out=sb[:, cb, :],
                        in_=dram[b].rearrange("(cb p) t -> p cb t", p=128)[:, cb, :])
                return sb

            def emit_loads(b):
                y_sb = emit_load(b, y2, py, "y")
                x_sb = emit_load(b, x2, px, "x")
                return x_sb, y_sb

            def emit_gn(src_sb, tag, spread=False):
                gn_tiles = [pgn.tile([128, 2, T], FP8, tag=tag, name=f"{tag}{kb2}")
                            for kb2 in range(2)]
                groupnorm(src_sb, gn_tiles, spread=spread)
                return gn_tiles

            # ---------------- projections ----------------
            def k_proj(gny):
                kq = [pkq.tile([128, 2, T], FP8, tag="kq", name=f"kq{q}") for q in range(2)]
                for ob in range(CB):
                    quad, i = ob // 2, ob % 2
                    for th in range(NT):
                        psk = ps_proj.tile([128, 512], F32, tag="mm")
                        for kb2 in range(2):
                            nc.tensor.matmul(
                                psk,
                                wk_sb[:, kb2, :, ob * 128:(ob + 1) * 128],
                                gny[kb2][:, :, th * 512:(th + 1) * 512],
                                start=(kb2 == 0), stop=(kb2 == 1),
                                perf_mode=PM.DoubleRow)
                        nc.scalar.activation(
                            out=kq[quad][:, i, th * 512:(th + 1) * 512],
                            in_=psk, func=AF.Identity,
                            bias=bk_sb[:, ob:ob + 1], scale=1.0)
                return kq

            def q_proj(gnx, q_pads):
                for ob in range(CB):
                    sub = ob % 2
                    h0, h1 = 2 * ob, 2 * ob + 1
                    for th in range(NT):
                        psq = ps_proj.tile([128, 512], F32, tag="mm")
                        for kb2 in range(2):
                            nc.tensor.matmul(
                                psq,
                                wq_sb[:, kb2, :, ob * 128:(ob + 1) * 128],
                                gnx[kb2][:, :, th * 512:(th + 1) * 512],
                                start=(kb2 == 0), stop=(kb2 == 1),
                                perf_mode=PM.DoubleRow)
                        nc.scalar.activation(
                            out=q_pads[h0][0:64, sub, th * 512:(th + 1) * 512],
                            in_=psq[0:64, :], func=AF.Identity,
                            bias=bq_sb[0:64, ob:ob + 1], scale=1.0)
                        nc.scalar.activation(
                            out=q_pads[h1][64:128, sub, th * 512:(th + 1) * 512],
                            in_=psq[64:128, :], func=AF.Identity,
                            bias=bq_sb[64:128, ob:ob + 1], scale=1.0)

            def v_proj(gny, b):
                # fp8 vt for chunks that are Act on at least one head parity;
                # bf16 vt for sts of chunks that are DVE on some parity.
                eng_b = EXP_ENG_B[b]
                need8 = {k for v in eng_b.values() for k in range(NK) if v[k] == 'A'}
                vt8 = {k: pvt8.tile([128, 2, NH, CH + 2], FP8, tag="vt8", name=f"vt8_{k}")
                       for k in need8}
                spill = {k for v in eng_b.values() for k in range(NK) if v[k] != 'A'}
                vt16 = {st: pvt16.tile([128, NH, CH + 2], BF16, tag="vt16", name=f"vt16_{st}")
                        for st in range(ST) if st // 2 in spill}
                for tt in range(ST):
                    psv = ps_proj.tile([128, 512], F32, tag="mm")
                    for kb2 in range(2):
                        nc.tensor.matmul(
                            psv,
                            gny[kb2][:, :, tt * 128:(tt + 1) * 128],
                            wv_sb[:, kb2, :, :],
                            start=(kb2 == 0), stop=(kb2 == 1),
                            perf_mode=PM.DoubleRow)
                    if tt // 2 in vt8:
                        dst = vt8[tt // 2]
                        nc.vector.tensor_tensor(
                            out=dst[:, tt % 2, :, 0:CH],
                            in0=psv.rearrange("p (h c) -> p h c", h=NH),
                            in1=bv_sb, op=OP.add)
                        nc.vector.tensor_copy(dst[:, tt % 2, :, CH:CH + 2], vcap8_sb)
                    if tt in vt16:
                        dst = vt16[tt]
                        nc.vector.tensor_tensor(
                            out=dst[:, :, 0:CH],
                            in0=psv.rearrange("p (h c) -> p h c", h=NH),
                            in1=bv_sb, op=OP.add)
                        nc.vector.tensor_copy(dst[:, :, CH:CH + 2], vcap16_sb)
                return vt8, vt16

            # ---------------- attention ----------------
            def attention_head(bctx, b, h, a_sb):
                kq = bctx["kq"]
                vt8, vt16 = bctx["vt"]
                q_pads = q_pad_sets[b % 2]
                quad = h // 4
                eng_k = EXP_ENG_B[b][h % 2]
                psa = ps_a.tile([CH + 2, T], F32, tag="psa")
                last_k = NK - 1
                wtiles = {}
                for k in range(NK):
                    if eng_k[k] == 'A':
                        wtiles[k] = pw8.tile([128, 2, T], FP8, tag="w8", name=f"w8_{h}_{k}")
                    else:
                        wtiles[k] = pw16.tile([128, 2, T], BF16, tag="w16", name=f"w16_{h}_{k}")
                # QK + exp; alternate consumer engines for pipeline depth
                inter = []
                dd = [k for k in range(NK) if eng_k[k] == 'D']
                aa = [k for k in range(NK) if eng_k[k] == 'A']
                if b == BPC - 1:
                    # drain batch: stream all Act tiles first, DVE tile last
                    inter = aa + dd
                else:
                    while dd or aa:
                        if dd:
                            inter.append(dd.pop(0))
                        if aa:
                            inter.append(aa.pop(0))
                for stp in range(2):
                    for k in inter:
                        st = 2 * k + stp
                        wts = wtiles[k]
                        scores = ps_sc.tile([128, T], F32, tag="sc")
                        for th in range(NT):
                            nc.tensor.matmul(
                                scores[:, th * 512:(th + 1) * 512],
                                kq[quad][:, :, st * 128:(st + 1) * 128],
                                q_pads[h][:, :, th * 512:(th + 1) * 512],
                                start=True, stop=True,
                                perf_mode=PM.DoubleRow)
                        if eng_k[k] == 'A':
                            nc.scalar.activation(
                                out=wts[:, stp, :], in_=scores,
                                func=AF.Exp, scale=QK_SCALE, bias=shift_sb)
                        else:
                            nc.vector.tensor_scalar(
                                out=wts[:, stp, :].bitcast(I16),
                                in0=scores, scalar1=SCH_A, scalar2=SCH_B,
                                op0=OP.mult, op1=OP.add)
                # AV burst (accumulate into psa)
                for k in range(NK):
                    wts = wtiles[k]
                    if eng_k[k] == 'A':
                        for th in range(NT):
                            nc.tensor.matmul(
                                psa[:, th * 512:(th + 1) * 512],
                                vt8[k][:, :, h, :],
                                wts[:, :, th * 512:(th + 1) * 512],
                                start=(k == 0), stop=(k == last_k),
                                perf_mode=PM.DoubleRow,
                                skip_group_check=True)
                    else:
                        for stp in range(2):
                            st = 2 * k + stp
                            for th in range(NT):
                                nc.tensor.matmul(
                                    psa[:, th * 512:(th + 1) * 512],
                                    vt16[st][:, h, :],
                                    wts[:, stp, th * 512:(th + 1) * 512],
                                    start=(k == 0 and stp == 0),
                                    stop=(k == last_k and stp == 1),
                                    skip_group_check=True)
                # ---- softmax denominator + normalize into a_sb ----
                rows = slice((h % 2) * 64, (h % 2) * 64 + 64)
                cbh = h // 2
                dcp = pr0.tile([CH + 1, T], F32, tag="dcp", name="dcp")
                nc.vector.tensor_copy(dcp[CH:CH + 1, :], psa[CH:CH + 1, :])
                r0 = pr0.tile([1, T], F32, tag="r0", name="r0")
                nc.sync.dma_start(out=r0, in_=dcp[CH:CH + 1, :])
                r0r = pr0.tile([1, T], F32, tag="r0r", name="r0r")
                nc.vector.reciprocal_approx_fast(out=r0r, in_=r0)
                if DEBUG and b == 0 and h == 0:
                    nc.sync.dma_start(out=dbg_r[:, :], in_=r0r)
                    for kk in range(NK):
                        wtmp = pdbg.tile([128, 2, T], F32, tag="wdbg", name=f"wdbg{kk}")
                        nc.vector.tensor_copy(wtmp.rearrange("p a b -> p (a b)"), wtiles[kk].rearrange("p a b -> p (a b)"))
                        nc.sync.dma_start(out=dbg_w[kk], in_=wtmp)
                rbc = prbc.tile([64, T], F32, tag="rbc")
                nc.gpsimd.partition_broadcast(rbc, r0r)
                nc.vector.tensor_tensor(
                    out=a_sb[rows, cbh, :],
                    in0=psa[0:CH, :],
                    in1=rbc, op=OP.mult)

            # ---------------- output projection ----------------
            def p_proj_ob(b, a_sb, x_sb, ob):
                for th in range(NT):
                    psh = ps_proj.tile([128, 512], F32, tag="mm")
                    for kb in range(CB):
                        nc.tensor.matmul(
                            psh,
                            wp_sb[:, kb, ob * 128:(ob + 1) * 128],
                            a_sb[:, kb, th * 512:(th + 1) * 512],
                            start=(kb == 0), stop=False,
                            skip_group_check=True)
                    # residual: psh += I @ x (bf16 identity, exact)
                    nc.tensor.matmul(
                        psh, ident16_sb,
                        x_sb[:, ob, th * 512:(th + 1) * 512],
                        start=False, stop=True,
                        skip_group_check=True)
                    ost = post.tile([128, 512], F32, tag="ost")
                    nc.scalar.activation(
                        out=ost, in_=psh, func=AF.Identity,
                        bias=bp_sb[:, ob:ob + 1], scale=1.0)
                    deng = nc.sync if th == 0 else nc.scalar
                    deng.dma_start(
                        out=out_d[b].rearrange("(cb p) t -> p cb t", p=128)[:, ob, th * 512:(th + 1) * 512],
                        in_=ost)

            # ---------------- batch pipeline ----------------
            bctxs = [dict() for _ in range(BPC)]
            y0 = emit_load(0, y2, py, "y")
            emit_small_consts()
            x0 = emit_load(0, x2, px, "x")
            bctxs[0]["x"] = x0
            emit_weight_loads()
            with tc.high_priority(10**6):
                bctxs[0]["gny"] = emit_gn(y0, "gny", spread=True)
                bctxs[0]["gnx"] = emit_gn(x0, "gnx", spread=True)
            emit_qpad_memsets(0)
            with tc.high_priority(10**6):
                bctxs[0]["kq"] = k_proj(bctxs[0]["gny"])
                bctxs[0]["vt"] = v_proj(bctxs[0]["gny"], 0)
                q_proj(bctxs[0]["gnx"], q_pad_sets[0])
            emit_qpad_memsets(1)
            if DEBUG:
                for kb2 in range(2):
                    gtmp = pdbg.tile([128, 2, T], F32, tag="wdbg", name=f"g{kb2}")
                    nc.vector.tensor_copy(gtmp.rearrange("p a b -> p (a b)"), bctxs[0]["gnx"][kb2].rearrange("p a b -> p (a b)"))
                    nc.sync.dma_start(out=dbg_gnx[kb2], in_=gtmp)
                    ktmp = pdbg.tile([128, 2, T], F32, tag="wdbg", name=f"k{kb2}")
                    nc.vector.tensor_copy(ktmp.rearrange("p a b -> p (a b)"), bctxs[0]["kq"][kb2].rearrange("p a b -> p (a b)"))
                    nc.sync.dma_start(out=dbg_kq[kb2], in_=ktmp)
                    qtmp = pdbg.tile([128, 2, T], F32, tag="wdbg", name=f"q{kb2}")
                    nc.vector.tensor_copy(qtmp.rearrange("p a b -> p (a b)"), q_pad_sets[0][kb2].rearrange("p a b -> p (a b)"))
                    nc.sync.dma_start(out=dbg_qp[kb2], in_=qtmp)

            prev = None  # (b, a_sb, x_sb) of the previous batch, p-proj pending
            for b in range(BPC):
                bctx = bctxs[b]
                nb = bctxs[b + 1] if b + 1 < BPC else None
                a_sb = pa.tile([128, CB, T], F32R, tag="a")
                for h in range(NH):
                    # attention stream outranks injected background work in
                    # the scheduler's priority heap (relative order preserved)
                    with tc.high_priority(10**6):
                        attention_head(bctx, b, h, a_sb)
                    if prev is not None and h < CB:
                        p_proj_ob(prev[0], prev[1], prev[2], h)
                        if h == CB - 1:
                            prev = None
                    if nb is not None:
                        if h == 0:
                            nxy = emit_loads(b + 1)
                            nb["x"] = nxy[0]
                            nb["_y"] = nxy[1]
                        elif h == 2:
                            nb["gny"] = emit_gn(nb["_y"], "gny")
                        elif h == 3:
                            nb["gnx"] = emit_gn(nb["x"], "gnx")
                        elif h == 4:
                            nb["kq"] = k_proj(nb["gny"])
                        elif h == 5:
                            nb["vt"] = v_proj(nb["gny"], b + 1)
                        elif h == 6:
                            q_proj(nb["gnx"], q_pad_sets[(b + 1) % 2])
                if DEBUG and b == 0:
                    nc.sync.dma_start(out=dbg_a.rearrange("p cb t -> p cb t"), in_=a_sb.bitcast(F32))
                prev = (b, a_sb, bctx["x"])
            for ob in range(CB):
                p_proj_ob(prev[0], prev[1], prev[2], ob)

    nc.finalize()
    return nc


_NC = None


def _get_nc():
    global _NC
    if _NC is None:
        _NC = _build()
    return _NC


def _prep_inputs(x, y, gn_w, gn_b, Wq, bq, Wkv, bkv, Wp, bp):
    import ml_dtypes
    FP8NP = ml_dtypes.float8_e4m3fn
    scale = CH ** -0.25
    idx_k = np.concatenate([np.arange(h * 2 * CH, h * 2 * CH + CH) for h in range(NH)])
    idx_v = np.concatenate([np.arange(h * 2 * CH + CH, (h + 1) * 2 * CH) for h in range(NH)])

    def dr_layout(wt):  # [C_in, C_out] -> [128, 2, 2, C_out]
        return np.ascontiguousarray(wt.reshape(2, 2, 128, C).transpose(2, 0, 1, 3))

    wq8 = dr_layout((Wq * (scale * WSCALE)).T).astype(FP8NP)
    wk8 = dr_layout((Wkv[idx_k] * (scale * WSCALE)).T).astype(FP8NP)
    wv8 = dr_layout((Wkv[idx_v] * WSCALE).T).astype(FP8NP)
    wpt = np.ascontiguousarray(Wp.T / WSCALE).astype(np.float32)

    def part_layout(v):
        return np.ascontiguousarray(v.reshape(CB, 128).T)

    bq_l = part_layout(bq * (scale * WSCALE))
    bk_l = part_layout(bkv[idx_k] * (scale * WSCALE))
    bp_l = part_layout(bp)
    gnw_l = part_layout(gn_w)
    gnb_l = part_layout(gn_b)
    bv = bkv[idx_v] * WSCALE
    bv_bc = np.broadcast_to(bv.reshape(1, NH, CH), (128, NH, CH)).copy().astype(np.float32)
    m1 = np.zeros((128, 128), np.float32)
    for g in range(128 // GSIZE):
        m1[g * GSIZE:(g + 1) * GSIZE, g * GSIZE:(g + 1) * GSIZE] = 1.0 / GSIZE
    vcap = np.zeros((128, NH, 2), np.float32)
    vcap[:, :, 0] = 1.0
    vcap8 = vcap.astype(FP8NP)
    vcap16 = vcap.astype(ml_dtypes.bfloat16)
    ident16 = np.eye(128, dtype=ml_dtypes.bfloat16)

    xf = x.reshape(B, C, T).astype(ml_dtypes.bfloat16)
    yf = y.reshape(B, C, T).astype(ml_dtypes.bfloat16)

    shared = {
        "wq8": wq8, "wk8": wk8, "wv8": wv8, "wpt": wpt,
        "bq_l": bq_l, "bk_l": bk_l, "bp_l": bp_l, "bv_bc": bv_bc,
        "gnw_l": gnw_l, "gnb_l": gnb_l, "m1": m1,
        "vcap8": vcap8, "vcap16": vcap16, "ident16": ident16,
    }
    in_maps = []
    for i in range(N_CORES):
        m = dict(shared)
        m["x2"] = np.ascontiguousarray(xf[i * BPC:(i + 1) * BPC])
        m["y2"] = np.ascontiguousarray(yf[i * BPC:(i + 1) * BPC])
        in_maps.append(m)
    return in_maps


def kernel(x, y, gn_w, gn_b, Wq, bq, Wkv, bkv, Wp, bp):
    args = [np.asarray(a, dtype=np.float32) for a in
            (x, y, gn_w, gn_b, Wq, bq, Wkv, bkv, Wp, bp)]
    in_maps = _prep_inputs(*args)
    nc = _get_nc()
    res = run_bass_kernel_spmd(nc, in_maps, core_ids=list(range(N_CORES)))
    out = np.empty((B, C, T), np.float32)
    for i in range(N_CORES):
        out[i * BPC:(i + 1) * BPC] = res.results[i]["out"]
    return out.reshape(B, C, H, W)


# revision 97
# speedup vs baseline: 1.0025x; 1.0025x over previous
"""AttentionBlock Trainium2 Bass kernel (fp8 DoubleRow redesign).

Data-parallel over batch: 16 batches / 8 cores = 2 per core.

Key design points vs the fp32r baseline:
- fp8e4m3 DoubleRow matmuls (contraction 256/instr, 0.5 cyc/row) for the
  q/k/v projections, QK scores, and AV; fp32r only for the output
  projection (accuracy).
- Weight pre-scaling: Wq,Wk x16 (compensated by exp scale 1/256), Wv x16
  (compensated by Wp/16) keeps fp8 weight entries out of the subnormal
  range.
- QK packs 4 heads' k-channels (256) into one DoubleRow contraction with
  per-head zero-padded q operands (zeros memset once, rows rewritten per
  batch).
- exp split across engines: Act does native exp -> fp8 wts; DVE/Pool do
  Schraudolph bf16 exp (int16 bit trick) for a subset of seq-chunks whose
  AV runs in bf16.
- softmax denominator from a ones-column appended to vT (row 64 of the AV
  psum); reciprocal on DVE, broadcast via a tiny PE matmul (ones x recip)
  into PSUM, normalize fused with the PSUM->SBUF move.
- groupnorm stats via bn_stats on bf16 inputs + block-diag matmul
  reduction; rsqrt via quake seed + 2 Newton steps (keeps Act on exp).
- x/y staged in bf16 (halves DMA); residual add reuses the staged x.
"""
import os
import sys

sys.path.insert(0, "/opt/trn_rl_repo")

import numpy as np

import concourse.bacc as bacc
import concourse.bass as bass
import concourse.tile as tile
from concourse import mybir
from concourse.bass_utils import run_bass_kernel_spmd

F32 = mybir.dt.float32
F32R = mybir.dt.float32r
BF16 = mybir.dt.bfloat16
FP8 = mybir.dt.float8e4
I16 = mybir.dt.int16
I32 = mybir.dt.int32
AF = mybir.ActivationFunctionType
OP = mybir.AluOpType
PM = mybir.MatmulPerfMode

B, C, H, W = 16, 512, 32, 32
T = H * W              # 1024
NH = 8                 # heads
CH = C // NH           # 64
GROUPS = 32
GSIZE = C // GROUPS    # 16
EPS = 1e-5
N_CORES = 8
BPC = B // N_CORES     # batches per core
CB = C // 128          # 4 channel blocks
NT = T // 512          # 2 column halves
ST = T // 128          # 8 seq tiles of 128
NK = ST // 2           # 4 DoubleRow seq chunks of 256

WSCALE = 16.0          # fp8 weight pre-scale for Wq/Wk/Wv
QK_SCALE = 1.0 / (WSCALE * WSCALE)   # folded into exp
# softmax shift: wts = exp(logit - EXP_SHIFT); cancels in normalization and
# keeps the max weight well under the fp8e4m3 max (448)
EXP_SHIFT = 2.0
# Schraudolph bf16 exp: bits16 = x*(2^7/ln2)*QK_SCALE + (127*2^7 - c)
SCH_A = 184.66496580927726 * QK_SCALE
SCH_B = 16250.4 - EXP_SHIFT * 184.66496580927726

# exp engine per k-chunk (0..3), by (batch, head parity): 'A' = Act native
# exp (fp8 wts, DR AV), 'D' = DVE Schraudolph (bf16 wts, bf16 AV). GPSIMD
# can't touch PSUM, so Pool gets only SBUF work. Batch 1 has no background
# work to hide DVE latency behind, so it runs all-Act.
EXP_ENG_B = {
    0: {0: ['A', 'D', 'A', 'A'], 1: ['A', 'D', 'A', 'A']},
    1: {0: ['A', 'A', 'A', 'A'], 1: ['A', 'A', 'A', 'A']},
}


def _build():
    nc = bacc.Bacc(None, target_bir_lowering=False)

    x2 = nc.dram_tensor("x2", (BPC, C, T), BF16, kind="ExternalInput")
    y2 = nc.dram_tensor("y2", (BPC, C, T), BF16, kind="ExternalInput")
    wq8_d = nc.dram_tensor("wq8", (128, 2, 2, C), FP8, kind="ExternalInput")
    wk8_d = nc.dram_tensor("wk8", (128, 2, 2, C), FP8, kind="ExternalInput")
    wv8_d = nc.dram_tensor("wv8", (128, 2, 2, C), FP8, kind="ExternalInput")
    wpt = nc.dram_tensor("wpt", (C, C), F32, kind="ExternalInput")
    bq_l = nc.dram_tensor("bq_l", (128, CB), F32, kind="ExternalInput")
    bk_l = nc.dram_tensor("bk_l", (128, CB), F32, kind="ExternalInput")
    bp_l = nc.dram_tensor("bp_l", (128, CB), F32, kind="ExternalInput")
    bv_bc = nc.dram_tensor("bv_bc", (128, NH, CH), F32, kind="ExternalInput")
    gnw_l = nc.dram_tensor("gnw_l", (128, CB), F32, kind="ExternalInput")
    gnb_l = nc.dram_tensor("gnb_l", (128, CB), F32, kind="ExternalInput")
    m1 = nc.dram_tensor("m1", (128, 128), F32, kind="ExternalInput")
    vcap8_d = nc.dram_tensor("vcap8", (128, NH, 2), FP8, kind="ExternalInput")
    vcap16_d = nc.dram_tensor("vcap16", (128, NH, 2), BF16, kind="ExternalInput")
    ident16_d = nc.dram_tensor("ident16", (128, 128), BF16, kind="ExternalInput")
    out_d = nc.dram_tensor("out", (BPC, C, T), F32, kind="ExternalOutput")
    DEBUG = bool(int(os.environ.get("KERNEL_DEBUG", "0")))
    if DEBUG:
        dbg_gnx = nc.dram_tensor("dbg_gnx", (2, 128, 2, T), F32, kind="ExternalOutput")
        dbg_kq = nc.dram_tensor("dbg_kq", (2, 128, 2, T), F32, kind="ExternalOutput")
        dbg_qp = nc.dram_tensor("dbg_qp", (2, 128, 2, T), F32, kind="ExternalOutput")
        dbg_a = nc.dram_tensor("dbg_a", (128, CB, T), F32, kind="ExternalOutput")
        dbg_r = nc.dram_tensor("dbg_r", (1, T), F32, kind="ExternalOutput")
        dbg_w = nc.dram_tensor("dbg_w", (NK, 128, 2, T), F32, kind="ExternalOutput")

    with tile.TileContext(nc) as tc:
        from contextlib import ExitStack
        with ExitStack() as ctx:
            consts = ctx.enter_context(tc.tile_pool(name="consts", bufs=1))
            px = ctx.enter_context(tc.tile_pool(name="px", bufs=2))
            py = ctx.enter_context(tc.tile_pool(name="py", bufs=1))
            pgn = ctx.enter_context(tc.tile_pool(name="pgn", bufs=3))
            pkq = ctx.enter_context(tc.tile_pool(name="pkq", bufs=4))
            pvt8 = ctx.enter_context(tc.tile_pool(name="pvt8", bufs=7))
            pvt16 = ctx.enter_context(tc.tile_pool(name="pvt16", bufs=2))
            pw8 = ctx.enter_context(tc.tile_pool(name="pw8", bufs=6))
            pw16 = ctx.enter_context(tc.tile_pool(name="pw16", bufs=3))
            pa = ctx.enter_context(tc.tile_pool(name="pa", bufs=2))
            pr0 = ctx.enter_context(tc.tile_pool(name="pr0", bufs=1))
            pst = ctx.enter_context(tc.tile_pool(name="pst", bufs=4))
            post = ctx.enter_context(tc.tile_pool(name="post", bufs=5))
            prbc = ctx.enter_context(tc.tile_pool(name="prbc", bufs=2))
            pdbg = ctx.enter_context(tc.tile_pool(name="pdbg", bufs=1)) if bool(int(os.environ.get("KERNEL_DEBUG", "0"))) else None
            ps_proj = ctx.enter_context(tc.tile_pool(name="ps_proj", bufs=2, space="PSUM"))
            ps_sc = ctx.enter_context(tc.tile_pool(name="ps_sc", bufs=2, space="PSUM"))
            ps_a = ctx.enter_context(tc.tile_pool(name="ps_a", bufs=1, space="PSUM"))

            # ---------------- constants ----------------
            wq_sb = consts.tile([128, 2, 2, C], FP8, tag="wq")
            wk_sb = consts.tile([128, 2, 2, C], FP8, tag="wk")
            wv_sb = consts.tile([128, 2, 2, C], FP8, tag="wv")
            wp_sb = consts.tile([128, CB, C], F32R, tag="wp")

            def emit_weight_loads():
                nc.sync.dma_start(out=wk_sb, in_=wk8_d[:, :, :, :])
                nc.sync.dma_start(out=wv_sb, in_=wv8_d[:, :, :, :])
                nc.sync.dma_start(out=wq_sb, in_=wq8_d[:, :, :, :])
                nc.sync.dma_start(out=wp_sb, in_=wpt.rearrange("(kb p) o -> p kb o", p=128).bitcast(F32R))

            m1_sb = consts.tile([128, 128], F32, tag="m1")
            bq_sb = consts.tile([128, CB], F32, tag="bq")
            bk_sb = consts.tile([128, CB], F32, tag="bk")
            bp_sb = consts.tile([128, CB], F32, tag="bp")
            bv_sb = consts.tile([128, NH, CH], F32, tag="bv")
            gnw_sb = consts.tile([128, CB], F32, tag="gnw")
            gnb_sb = consts.tile([128, CB], F32, tag="gnb")
            vcap8_sb = consts.tile([128, NH, 2], FP8, tag="vcap8")
            vcap16_sb = consts.tile([128, NH, 2], BF16, tag="vcap16")
            ident16_sb = consts.tile([128, 128], BF16, tag="ident16")
            magic_sb = consts.tile([128, CB], I32, tag="magic")
            nc.vector.memset(magic_sb, 0x5f3759df)
            shift_sb = consts.tile([128, 1], F32, tag="shift")
            nc.vector.memset(shift_sb, -EXP_SHIFT)
            warm = consts.tile([1, 1], F32, tag="warm")
            nc.vector.memset(warm, 0.0)
            nc.scalar.activation(out=warm, in_=warm, func=AF.Exp)

            def emit_small_consts():
                nc.sync.dma_start(out=m1_sb, in_=m1[:, :])
                nc.sync.dma_start(out=gnw_sb, in_=gnw_l[:, :])
                nc.sync.dma_start(out=gnb_sb, in_=gnb_l[:, :])
                nc.sync.dma_start(out=bk_sb, in_=bk_l[:, :])
                nc.sync.dma_start(out=bq_sb, in_=bq_l[:, :])
                nc.sync.dma_start(out=bv_sb, in_=bv_bc[:, :, :])
                nc.sync.dma_start(out=bp_sb, in_=bp_l[:, :])
                nc.sync.dma_start(out=vcap8_sb, in_=vcap8_d[:, :, :])
                nc.sync.dma_start(out=vcap16_sb, in_=vcap16_d[:, :, :])
                nc.sync.dma_start(out=ident16_sb, in_=ident16_d[:, :])

            # persistent zero-padded q operands: [128, 2, T] fp8 per head,
            # double-buffered by batch parity (avoids WAR with the next
            # batch's q_proj); head h occupies rows (h%2)*64.. at sub
            # (h//2)%2; zeros persist.
            q_pad_sets = [
                [consts.tile([128, 2, T], FP8, tag=f"qpad{g}_{h}", name=f"qpad{g}_{h}")
                 for h in range(NH)]
                for g in range(2)]

            def emit_qpad_memsets(g):
                # Pool only (idle engine), chunked so ready work can slip in
                for qp in q_pad_sets[g]:
                    qpf = qp.rearrange("p a b -> p (a b)")
                    for c in range(4):
                        nc.gpsimd.memset(qpf[:, c * 512:(c + 1) * 512], 0.0)

            # ---------------- groupnorm ----------------
            def groupnorm(src_sb, gn_tiles, spread=False):
                """src_sb: [128, CB, T] bf16. gn_tiles: 2 x [128, 2, T] fp8."""
                mv = pst.tile([128, CB, 2], F32, tag="mv")
                stats6 = pst.tile([128, 2, 6], F32, tag="stats6")
                for cb in range(CB):
                    for c2 in range(2):
                        nc.vector.bn_stats(
                            out=stats6[:, c2, :],
                            in_=src_sb[:, cb, c2 * 512:(c2 + 1) * 512])
                    nc.vector.bn_aggr(out=mv[:, cb, :], in_=stats6)
                musq = pst.tile([128, 4], F32, tag="musq")
                nc.vector.tensor_tensor(out=musq, in0=mv[:, :, 0], in1=mv[:, :, 0], op=OP.mult)
                nc.vector.tensor_tensor(out=mv[:, :, 1], in0=musq, in1=mv[:, :, 1], op=OP.add)
                psgt = ps_proj.tile([128, 512], F32, tag="mm", name="psgt")
                psg = psgt[:, 0:8]
                nc.tensor.matmul(psg, m1_sb, mv.rearrange("p a b -> p (a b)"), start=True, stop=True)
                gsb = pst.tile([128, 8], F32, tag="gsb")
                nc.vector.tensor_copy(gsb, psg)
                tmp4 = pst.tile([128, 4], F32, tag="tmp4")
                nc.vector.tensor_tensor(out=tmp4, in0=gsb[:, 0::2], in1=gsb[:, 0::2], op=OP.mult)
                vv = pst.tile([128, 4], F32, tag="vv")
                nc.vector.scalar_tensor_tensor(
                    out=vv, in0=gsb[:, 1::2], scalar=EPS, in1=tmp4,
                    op0=OP.add, op1=OP.subtract)
                bsh = pst.tile([128, 4], I32, tag="bsh")
                nc.vector.tensor_scalar(
                    out=bsh, in0=vv.bitcast(I32), scalar1=1, scalar2=None,
                    op0=OP.logical_shift_right)
                nc.vector.tensor_tensor(out=tmp4.bitcast(I32), in0=magic_sb, in1=bsh, op=OP.subtract)
                nrt = pst.tile([128, 4], F32, tag="nrt")
                for _ in range(2):
                    nc.vector.tensor_tensor(out=nrt, in0=tmp4, in1=tmp4, op=OP.mult)
                    nc.vector.scalar_tensor_tensor(
                        out=nrt, in0=nrt, scalar=-0.5, in1=vv, op0=OP.mult, op1=OP.mult)
                    nc.vector.scalar_tensor_tensor(
                        out=tmp4, in0=nrt, scalar=1.5, in1=tmp4, op0=OP.add, op1=OP.mult)
                ab = pst.tile([128, 8], F32, tag="ab")
                nc.vector.tensor_tensor(out=ab[:, 0:4], in0=tmp4, in1=gnw_sb, op=OP.mult)
                tmp4b = pst.tile([128, 4], F32, tag="tmp4b")
                nc.vector.tensor_tensor(out=tmp4b, in0=gsb[:, 0::2], in1=ab[:, 0:4], op=OP.mult)
                nc.vector.tensor_tensor(out=ab[:, 4:8], in0=gnb_sb, in1=tmp4b, op=OP.subtract)
                for cb in range(CB):
                    kb2, i = cb // 2, cb % 2
                    for th in range(NT):
                        eng = nc.gpsimd if not spread else \
                            (nc.vector, nc.gpsimd)[(cb * NT + th) % 2]
                        eng.tensor_scalar(
                            out=gn_tiles[kb2][:, i, th * 512:(th + 1) * 512],
                            in0=src_sb[:, cb, th * 512:(th + 1) * 512],
                            scalar1=ab[:, cb:cb + 1], scalar2=ab[:, 4 + cb:5 + cb],
                            op0=OP.mult, op1=OP.add)

            def emit_load(b, dram, pool, tag):
                sb = pool.tile([128, CB, T], BF16, tag=tag, name=tag)
                for cb in range(CB):
                    deng = nc.sync if cb % 2 == 0 else nc.scalar
                    deng.dma_start(
                        out=sb[:, cb, :],
                        in_=dram[b].rearrange("(cb p) t -> p cb t", p=128)[:, cb, :])
                return sb

            def emit_loads(b):
                y_sb = emit_load(b, y2, py, "y")
                x_sb = emit_load(b, x2, px, "x")
                return x_sb, y_sb

            def emit_gn(src_sb, tag, spread=False):
                gn_tiles = [pgn.tile([128, 2, T], FP8, tag=tag, name=f"{tag}{kb2}")
                            for kb2 in range(2)]
                groupnorm(src_sb, gn_tiles, spread=spread)
                return gn_tiles

            # ---------------- projections ----------------
            def k_proj(gny):
                kq = [pkq.tile([128, 2, T], FP8, tag="kq", name=f"kq{q}") for q in range(2)]
                for ob in range(CB):
                    quad, i = ob // 2, ob % 2
                    for th in range(NT):
                        psk = ps_proj.tile([128, 512], F32, tag="mm")
                        for kb2 in range(2):
                            nc.tensor.matmul(
                                psk,
                                wk_sb[:, kb2, :, ob * 128:(ob + 1) * 128],
                                gny[kb2][:, :, th * 512:(th + 1) * 512],
                                start=(kb2 == 0), stop=(kb2 == 1),
                                perf_mode=PM.DoubleRow)
                        nc.scalar.activation(
                            out=kq[quad][:, i, th * 512:(th + 1) * 512],
                            in_=psk, func=AF.Identity,
                            bias=bk_sb[:, ob:ob + 1], scale=1.0)
                return kq

            def q_proj(gnx, q_pads):
                for ob in range(CB):
                    sub = ob % 2
                    h0, h1 = 2 * ob, 2 * ob + 1
                    for th in range(NT):
                        psq = ps_proj.tile([128, 512], F32, tag="mm")
                        for kb2 in range(2):
                            nc.tensor.matmul(
                                psq,
                                wq_sb[:, kb2, :, ob * 128:(ob + 1) * 128],
                                gnx[kb2][:, :, th * 512:(th + 1) * 512],
                                start=(kb2 == 0), stop=(kb2 == 1),
                                perf_mode=PM.DoubleRow)
                        nc.scalar.activation(
                            out=q_pads[h0][0:64, sub, th * 512:(th + 1) * 512],
                            in_=psq[0:64, :], func=AF.Identity,
                            bias=bq_sb[0:64, ob:ob + 1], scale=1.0)
                        nc.scalar.activation(
                            out=q_pads[h1][64:128, sub, th * 512:(th + 1) * 512],
                            in_=psq[64:128, :], func=AF.Identity,
                            bias=bq_sb[64:128, ob:ob + 1], scale=1.0)

            def v_proj(gny, b):
                # fp8 vt for chunks that are Act on at least one head parity;
                # bf16 vt for sts of chunks that are DVE on some parity.
                eng_b = EXP_ENG_B[b]
                need8 = {k for v in eng_b.values() for k in range(NK) if v[k] == 'A'}
                vt8 = {k: pvt8.tile([128, 2, NH, CH + 2], FP8, tag="vt8", name=f"vt8_{k}")
                       for k in need8}
                spill = {k for v in eng_b.values() for k in range(NK) if v[k] != 'A'}
                vt16 = {st: pvt16.tile([128, NH, CH + 2], BF16, tag="vt16", name=f"vt16_{st}")
                        for st in range(ST) if st // 2 in spill}
                for tt in range(ST):
                    psv = ps_proj.tile([128, 512], F32, tag="mm")
                    for kb2 in range(2):
                        nc.tensor.matmul(
                            psv,
                            gny[kb2][:, :, tt * 128:(tt + 1) * 128],
                            wv_sb[:, kb2, :, :],
                            start=(kb2 == 0), stop=(kb2 == 1),
                            perf_mode=PM.DoubleRow)
                    if tt // 2 in vt8:
                        dst = vt8[tt // 2]
                        nc.vector.tensor_tensor(
                            out=dst[:, tt % 2, :, 0:CH],
                            in0=psv.rearrange("p (h c) -> p h c", h=NH),
                            in1=bv_sb, op=OP.add)
                        nc.vector.tensor_copy(dst[:, tt % 2, :, CH:CH + 2], vcap8_sb)
                    if tt in vt16:
                        dst = vt16[tt]
                        nc.vector.tensor_tensor(
                            out=dst[:, :, 0:CH],
                            in0=psv.rearrange("p (h c) -> p h c", h=NH),
                            in1=bv_sb, op=OP.add)
                        nc.vector.tensor_copy(dst[:, :, CH:CH + 2], vcap16_sb)
                return vt8, vt16

            # ---------------- attention ----------------
            def attention_head(bctx, b, h, a_sb):
                kq = bctx["kq"]
                vt8, vt16 = bctx["vt"]
                q_pads = q_pad_sets[b % 2]
                quad = h // 4
                eng_k = EXP_ENG_B[b][h % 2]
                psa = ps_a.tile([CH + 2, T], F32, tag="psa")
                last_k = NK - 1
                wtiles = {}
                for k in range(NK):
                    if eng_k[k] == 'A':
                        wtiles[k] = pw8.tile([128, 2, T], FP8, tag="w8", name=f"w8_{h}_{k}")
                    else:
                        wtiles[k] = pw16.tile([128, 2, T], BF16, tag="w16", name=f"w16_{h}_{k}")
                # QK + exp; alternate consumer engines for pipeline depth
                inter = []
                dd = [k for k in range(NK) if eng_k[k] == 'D']
                aa = [k for k in range(NK) if eng_k[k] == 'A']
                if b == BPC - 1:
                    # drain batch: stream all Act tiles first, DVE tile last
                    inter = aa + dd
                else:
                    while dd or aa:
                        if dd:
                            inter.append(dd.pop(0))
                        if aa:
                            inter.append(aa.pop(0))
                for stp in range(2):
                    for k in inter:
                        st = 2 * k + stp
                        wts = wtiles[k]
                        scores = ps_sc.tile([128, T], F32, tag="sc")
                        for th in range(NT):
                            nc.tensor.matmul(
                                scores[:, th * 512:(th + 1) * 512],
                                kq[quad][:, :, st * 128:(st + 1) * 128],
                                q_pads[h][:, :, th * 512:(th + 1) * 512],
                                start=True, stop=True,
                                perf_mode=PM.DoubleRow)
                        if eng_k[k] == 'A':
                            nc.scalar.activation(
                                out=wts[:, stp, :], in_=scores,
                                func=AF.Exp, scale=QK_SCALE, bias=shift_sb)
                        else:
                            nc.vector.tensor_scalar(
                                out=wts[:, stp, :].bitcast(I16),
                                in0=scores, scalar1=SCH_A, scalar2=SCH_B,
                                op0=OP.mult, op1=OP.add)
                # AV burst (accumulate into psa)
                for k in range(NK):
                    wts = wtiles[k]
                    if eng_k[k] == 'A':
                        for th in range(NT):
                            nc.tensor.matmul(
                                psa[:, th * 512:(th + 1) * 512],
                                vt8[k][:, :, h, :],
                                wts[:, :, th * 512:(th + 1) * 512],
                                start=(k == 0), stop=(k == last_k),
                                perf_mode=PM.DoubleRow,
                                skip_group_check=True)
                    else:
                        for stp in range(2):
                            st = 2 * k + stp
                            for th in range(NT):
                                nc.tensor.matmul(
                                    psa[:, th * 512:(th + 1) * 512],
                                    vt16[st][:, h, :],
                                    wts[:, stp, th * 512:(th + 1) * 512],
                                    start=(k == 0 and stp == 0),
                                    stop=(k == last_k and stp == 1),
                                    skip_group_check=True)
                # ---- softmax denominator + normalize into a_sb ----
                rows = slice((h % 2) * 64, (h % 2) * 64 + 64)
                cbh = h // 2
                dcp = pr0.tile([CH + 1, T], F32, tag="dcp", name="dcp")
                nc.vector.tensor_copy(dcp[CH:CH + 1, :], psa[CH:CH + 1, :])
                r0 = pr0.tile([1, T], F32, tag="r0", name="r0")
                nc.sync.dma_start(out=r0, in_=dcp[CH:CH + 1, :])
                r0r = pr0.tile([1, T], F32, tag="r0r", name="r0r")
                nc.vector.reciprocal_approx_fast(out=r0r, in_=r0)
                if DEBUG and b == 0 and h == 0:
                    nc.sync.dma_start(out=dbg_r[:, :], in_=r0r)
                    for kk in range(NK):
                        wtmp = pdbg.tile([128, 2, T], F32, tag="wdbg", name=f"wdbg{kk}")
                        nc.vector.tensor_copy(wtmp.rearrange("p a b -> p (a b)"), wtiles[kk].rearrange("p a b -> p (a b)"))
                        nc.sync.dma_start(out=dbg_w[kk], in_=wtmp)
                rbc = prbc.tile([64, T], F32, tag="rbc")
                nc.gpsimd.partition_broadcast(rbc, r0r)
                nc.vector.tensor_tensor(
                    out=a_sb[rows, cbh, :],
                    in0=psa[0:CH, :],
                    in1=rbc, op=OP.mult)

            # ---------------- output projection ----------------
            def p_proj_ob(b, a_sb, x_sb, ob):
                for th in range(NT):
                    psh = ps_proj.tile([128, 512], F32, tag="mm")
                    for kb in range(CB):
                        nc.tensor.matmul(
                            psh,
                            wp_sb[:, kb, ob * 128:(ob + 1) * 128],
                            a_sb[:, kb, th * 512:(th + 1) * 512],
                            start=(kb == 0), stop=False,
                            skip_group_check=True)
                    # residual: psh += I @ x (bf16 identity, exact)
                    nc.tensor.matmul(
                        psh, ident16_sb,
                        x_sb[:, ob, th * 512:(th + 1) * 512],
                        start=False, stop=True,
                        skip_group_check=True)
                    ost = post.tile([128, 512], F32, tag="ost")
                    nc.scalar.activation(
                        out=ost, in_=psh, func=AF.Identity,
                        bias=bp_sb[:, ob:ob + 1], scale=1.0)
                    deng = nc.sync if th == 0 else nc.scalar
                    deng.dma_start(
                        out=out_d[b].rearrange("(cb p) t -> p cb t", p=128)[:, ob, th * 512:(th + 1) * 512],
                        in_=ost)

            # ---------------- batch pipeline ----------------
            bctxs = [dict() for _ in range(BPC)]
            y0 = emit_load(0, y2, py, "y")
            emit_small_consts()
            x0 = emit_load(0, x2, px, "x")
            bctxs[0]["x"] = x0
            emit_weight_loads()
            with tc.high_priority(10**6):
                bctxs[0]["gny"] = emit_gn(y0, "gny", spread=True)
                bctxs[0]["gnx"] = emit_gn(x0, "gnx", spread=True)
            emit_qpad_memsets(0)
            with tc.high_priority(10**6):
                bctxs[0]["kq"] = k_proj(bctxs[0]["gny"])
                bctxs[0]["vt"] = v_proj(bctxs[0]["gny"], 0)
                q_proj(bctxs[0]["gnx"], q_pad_sets[0])
            emit_qpad_memsets(1)
            if DEBUG:
                for kb2 in range(2):
                    gtmp = pdbg.tile([128, 2, T], F32, tag="wdbg", name=f"g{kb2}")
                    nc.vector.tensor_copy(gtmp.rearrange("p a b -> p (a b)"), bctxs[0]["gnx"][kb2].rearrange("p a b -> p (a b)"))
                    nc.sync.dma_start(out=dbg_gnx[kb2], in_=gtmp)
                    ktmp = pdbg.tile([128, 2, T], F32, tag="wdbg", name=f"k{kb2}")
                    nc.vector.tensor_copy(ktmp.rearrange("p a b -> p (a b)"), bctxs[0]["kq"][kb2].rearrange("p a b -> p (a b)"))
                    nc.sync.dma_start(out=dbg_kq[kb2], in_=ktmp)
                    qtmp = pdbg.tile([128, 2, T], F32, tag="wdbg", name=f"q{kb2}")
                    nc.vector.tensor_copy(qtmp.rearrange("p a b -> p (a b)"), q_pad_sets[0][kb2].rearrange("p a b -> p (a b)"))
                    nc.sync.dma_start(out=dbg_qp[kb2], in_=qtmp)

            prev = None  # (b, a_sb, x_sb) of the previous batch, p-proj pending
            for b in range(BPC):
                bctx = bctxs[b]
                nb = bctxs[b + 1] if b + 1 < BPC else None
                a_sb = pa.tile([128, CB, T], F32R, tag="a")
                for h in range(NH):
                    # attention stream outranks injected background work in
                    # the scheduler's priority heap (relative order preserved)
                    with tc.high_priority(10**6):
                        attention_head(bctx, b, h, a_sb)
                    if prev is not None and h < CB:
                        p_proj_ob(prev[0], prev[1], prev[2], h)
                        if h == CB - 1:
                            prev = None
                    if nb is not None:
                        if h == 0:
                            nxy = emit_loads(b + 1)
                            nb["x"] = nxy[0]
                            nb["_y"] = nxy[1]
                        elif h == 2:
                            nb["gny"] = emit_gn(nb["_y"], "gny")
                        elif h == 3:
                            nb["gnx"] = emit_gn(nb["x"], "gnx")
                        elif h == 4:
                            nb["kq"] = k_proj(nb["gny"])
                        elif h == 5:
                            nb["vt"] = v_proj(nb["gny"], b + 1)
                        elif h == 6:
                            q_proj(nb["gnx"], q_pad_sets[(b + 1) % 2])
                if DEBUG and b == 0:
                    nc.sync.dma_start(out=dbg_a.rearrange("p cb t -> p cb t"), in_=a_sb.bitcast(F32))
                prev = (b, a_sb, bctx["x"])
            for ob in range(CB):
                p_proj_ob(prev[0], prev[1], prev[2], ob)

    nc.finalize()
    return nc


_NC = None


def _get_nc():
    global _NC
    if _NC is None:
        _NC = _build()
    return _NC


def _prep_inputs(x, y, gn_w, gn_b, Wq, bq, Wkv, bkv, Wp, bp):
    import ml_dtypes
    FP8NP = ml_dtypes.float8_e4m3fn
    scale = CH ** -0.25
    idx_k = np.concatenate([np.arange(h * 2 * CH, h * 2 * CH + CH) for h in range(NH)])
    idx_v = np.concatenate([np.arange(h * 2 * CH + CH, (h + 1) * 2 * CH) for h in range(NH)])

    def dr_layout(wt):  # [C_in, C_out] -> [128, 2, 2, C_out]
        return np.ascontiguousarray(wt.reshape(2, 2, 128, C).transpose(2, 0, 1, 3))

    wq8 = dr_layout((Wq * (scale * WSCALE)).T).astype(FP8NP)
    wk8 = dr_layout((Wkv[idx_k] * (scale * WSCALE)).T).astype(FP8NP)
    wv8 = dr_layout((Wkv[idx_v] * WSCALE).T).astype(FP8NP)
    wpt = np.ascontiguousarray(Wp.T / WSCALE).astype(np.float32)

    def part_layout(v):
        return np.ascontiguousarray(v.reshape(CB, 128).T)

    bq_l = part_layout(bq * (scale * WSCALE))
    bk_l = part_layout(bkv[idx_k] * (scale * WSCALE))
    bp_l = part_layout(bp)
    gnw_l = part_layout(gn_w)
    gnb_l = part_layout(gn_b)
    bv = bkv[idx_v] * WSCALE
    bv_bc = np.broadcast_to(bv.reshape(1, NH, CH), (128, NH, CH)).copy().astype(np.float32)
    m1 = np.zeros((128, 128), np.float32)
    for g in range(128 // GSIZE):
        m1[g * GSIZE:(g + 1) * GSIZE, g * GSIZE:(g + 1) * GSIZE] = 1.0 / GSIZE
    vcap = np.zeros((128, NH, 2), np.float32)
    vcap[:, :, 0] = 1.0
    vcap8 = vcap.astype(FP8NP)
    vcap16 = vcap.astype(ml_dtypes.bfloat16)
    ident16 = np.eye(128, dtype=ml_dtypes.bfloat16)

    xf = x.reshape(B, C, T).astype(ml_dtypes.bfloat16)
    yf = y.reshape(B, C, T).astype(ml_dtypes.bfloat16)

    shared = {
        "wq8": wq8, "wk8": wk8, "wv8": wv8, "wpt": wpt,
        "bq_l": bq_l, "bk_l": bk_l, "bp_l": bp_l, "bv_bc": bv_bc,
        "gnw_l": gnw_l, "gnb_l": gnb_l, "m1": m1,
        "vcap8": vcap8, "vcap16": vcap16, "ident16": ident16,
    }
    in_maps = []
    for i in range(N_CORES):
        m = dict(shared)
        m["x2"] = np.ascontiguousarray(xf[i * BPC:(i + 1) * BPC])
        m["y2"] = np.ascontiguousarray(yf[i * BPC:(i + 1) * BPC])
        in_maps.append(m)
    return in_maps


def kernel(x, y, gn_w, gn_b, Wq, bq, Wkv, bkv, Wp, bp):
    args = [np.asarray(a, dtype=np.float32) for a in
            (x, y, gn_w, gn_b, Wq, bq, Wkv, bkv, Wp, bp)]
    in_maps = _prep_inputs(*args)
    nc = _get_nc()
    res = run_bass_kernel_spmd(nc, in_maps, core_ids=list(range(N_CORES)))
    out = np.empty((B, C, T), np.float32)
    for i in range(N_CORES):
        out[i * BPC:(i + 1) * BPC] = res.results[i]["out"]
    return out.reshape(B, C, H, W)
